# revision 17
# baseline (speedup 1.0000x reference)
"""LoRA Linear kernel for 8x TRN2 NeuronCores (Bass/Tile).

Computes  y = x @ W^T + b + 2.0 * ((x @ A^T) @ B^T)   for
  x [4, 2048, 4096] f32, W [4096, 4096], b [4096], A [16, 4096], B [4096, 16].

Strategy (v2):
  - Data-parallel over tokens: 8192 tokens -> 1024 per core.
  - Stationary operand is the W o-tile [128d, 128o]; the moving operand
    sweeps tokens, so one Ldweights serves two N=512 matmuls (the baseline
    was 1 Ldweights per matmul, which exposed ~43ns per pair).
  - Output computed as [O, TC] per core (o on partitions); host transposes.
  - Bias is folded into the LoRA matmul: stationary [17, o128] =
    [2*B^T; b], moving [17, t512] = [xa^T; ones].  One K=17 matmul per
    (o-tile, t-chunk) finishes each PSUM accumulation group.
  - W is host-prepacked per o-tile so each DMA line is 8KB contiguous.
  - DMA queues: sync = x/A/Baug in, scalar = W stream in, vector = out.
"""

import os

import numpy as np
import ml_dtypes

_BF16 = ml_dtypes.bfloat16

# Problem constants (hardcoded per harness contract).
_B, _S, _D, _O, _R = 4, 2048, 4096, 4096, 16
_T = _B * _S          # 8192 tokens
_NCORES = 8
_TC = _T // _NCORES   # 1024 tokens per core

P = 128
DS = _D // P          # 32 contraction subtiles
NOT = _O // P         # 32 o-tiles
TCH = 512             # token chunk (moving N)
NCH = _TC // TCH      # 2 chunks per core
RA = _R + 1           # lora rows + bias row

_cache = {}

# Set by kernel() when KERNEL_TRACE=1; read by test.py for exec_time_ns.
LAST_RESULT = None


def _build_module():
    import concourse.bass as bass
    import concourse.bacc as bacc
    import concourse.mybir as mybir
    import concourse.tile as tile
    from concourse.bass import ts

    bf16 = mybir.dt.bfloat16
    f32 = mybir.dt.float32

    nc = bacc.Bacc("TRN2", target_bir_lowering=False, debug=False)
    x0_d = nc.dram_tensor("x0", [P, DS, TCH], bf16, kind="ExternalInput")
    x1_d = nc.dram_tensor("x1", [P, DS, TCH], bf16, kind="ExternalInput")
    Wp_d = nc.dram_tensor("Wp", [NOT * P, DS, P], bf16, kind="ExternalInput")
    ATp_d = nc.dram_tensor("ATp", [P, DS, _R], bf16, kind="ExternalInput")
    Baug_d = nc.dram_tensor("Baug", [RA, _O], bf16, kind="ExternalInput")
    ones_d = nc.dram_tensor("ones", [1, _TC], bf16, kind="ExternalInput")
    out_d = nc.dram_tensor("out", [_O, _TC], f32, kind="ExternalOutput")

    with tile.TileContext(nc) as tc:
        with (
            tc.tile_pool(name="const", bufs=1) as cpool,
            tc.tile_pool(name="wpool", bufs=6) as wpool,
            tc.tile_pool(name="opool", bufs=3) as opool,
            tc.tile_pool(name="ps_mm", bufs=2, space="PSUM") as ps_pool,
            tc.tile_pool(name="ps_xa", bufs=2, space="PSUM") as ps_xa_pool,
        ):
            AT_sb = cpool.tile([P, DS, _R], bf16)
            NG = 4           # ds-groups per x chunk 0 (fine-grained arrival)
            GD = DS // NG    # 8 ds per group
            x0g = [
                cpool.tile([P, GD, TCH], bf16, name=f"x0g{g}") for g in range(NG)
            ]
            x_sb1 = cpool.tile([P, DS, TCH], bf16)
            Baug_sb = cpool.tile([RA, _O], bf16)
            xa_aug = cpool.tile([RA, _TC], bf16)
            dum_w = cpool.tile([P, P], bf16)
            dum_x = cpool.tile([P, TCH], bf16)

            def x0mov(ds):
                return x0g[ds // GD][:, ds % GD, :]

            # Warm the PE HAM clock gate with dummy matmuls (zero inputs,
            # result never read) while the x DMA streams in.
            nc.vector.memset(dum_w[:], 0.0)
            nc.vector.memset(dum_x[:], 0.0)
            ps_dum = ps_xa_pool.tile([P, TCH], f32, name="ps_dum")
            for i in range(12):
                nc.tensor.matmul(
                    ps_dum[:], dum_w[:], dum_x[:], start=(i == 0), stop=(i == 11)
                )

            # x chunk 0 in 4 ds-group tiles alternating queues: the first xa
            # matmul only needs group 0 (~1MB), not the whole 8MB of x.
            H = GD // 2
            nc.sync.dma_start(AT_sb[:], ATp_d[:, :, :])
            for g in range(NG):
                q = nc.sync if g % 2 == 0 else nc.scalar
                q.dma_start(x0g[g][:], x0_d[:, g * GD : (g + 1) * GD, :])
            nc.sync.dma_start(x_sb1[:, 0 : DS // 2, :], x1_d[:, 0 : DS // 2, :])
            nc.scalar.dma_start(x_sb1[:, DS // 2 : DS, :], x1_d[:, DS // 2 : DS, :])
            nc.sync.dma_start(xa_aug[_R : _R + 1, :], ones_d[:, :])
            nc.sync.dma_start(Baug_sb[:], Baug_d[:, :])

            # xa^T[r, t] = sum_ds A^T[ds, r].T @ x^T[ds, t]
            for c in range(NCH):
                ps_xa = ps_xa_pool.tile([_R, TCH], f32)
                for ds in range(DS):
                    nc.tensor.matmul(
                        ps_xa[:],
                        AT_sb[:, ds, :],
                        x0mov(ds) if c == 0 else x_sb1[:, ds, :],
                        start=(ds == 0),
                        stop=(ds == DS - 1),
                    )
                nc.vector.tensor_copy(xa_aug[0:_R, ts(c, TCH)], ps_xa[:])

            for ot in range(NOT):
                Wt = wpool.tile([P, DS, P], bf16)
                nc.scalar.dma_start(Wt[:], Wp_d[ts(ot, P), :, :])
                ps = [
                    ps_pool.tile([P, TCH], f32, name=f"ps{c}") for c in range(NCH)
                ]
                for ds in range(DS):
                    for c in range(NCH):
                        nc.tensor.matmul(
                            ps[c][:],
                            Wt[:, ds, :],
                            x0mov(ds) if c == 0 else x_sb1[:, ds, :],
                            start=(ds == 0),
                            stop=False,
                        )
                # LoRA + bias: [2B^T; b][:, ot].T @ [xa^T; ones] , K=17
                for c in range(NCH):
                    nc.tensor.matmul(
                        ps[c][:],
                        Baug_sb[:, ts(ot, P)],
                        xa_aug[:, ts(c, TCH)],
                        start=False,
                        stop=True,
                    )
                # Quartered drains (ACT + DVE in parallel on different banks)
                # and out DMAs on alternating queues: stream results out
                # early so the end-of-kernel flush is short.
                QW = TCH // 2
                for c in range(NCH):
                    for h in range(2):
                        q = c * 2 + h
                        qt = opool.tile([P, QW], f32, name=f"ot_q{q}")
                        if h == 0:
                            nc.scalar.copy(qt[:], ps[c][:, ts(h, QW)])
                        else:
                            nc.vector.tensor_copy(qt[:], ps[c][:, ts(h, QW)])
                        dq = nc.sync if q % 2 == 0 else nc.scalar
                        dq.dma_start(out_d[ts(ot, P), q * QW : (q + 1) * QW], qt[:])

    _dedup_ldweights(nc, mybir)
    nc.compile()
    return nc


def _dedup_ldweights(nc, mybir):
    """Drop PE Ldweights that reload the stationary already in the array.

    The tile pass lowers every matmul to an Ldweights+Matmult pair even when
    consecutive matmuls share the stationary operand.  The redundant reload
    costs PE cycles (~46ns exposed per pair at N=512).  Weights persist in
    the array across Matmults, so a back-to-back identical Ldweights with no
    semaphore activity is dead.
    """
    n_drop = 0
    for fn in nc.m.functions:
        for blk in fn.blocks:
            insts = blk.instructions
            new = []
            prev_key = None
            for inst in insts:
                if inst.engine != mybir.EngineType.PE:
                    new.append(inst)
                    continue
                if isinstance(inst, mybir.InstLdweights):
                    key = str(inst.ins[0])
                    if (
                        key == prev_key
                        and not inst.has_wait()
                        and not inst.has_update()
                    ):
                        n_drop += 1
                        continue
                    prev_key = key
                elif isinstance(inst, mybir.InstMatmult):
                    if inst.is_transpose:
                        prev_key = None
                elif isinstance(inst, mybir.InstEventSemaphore):
                    pass
                else:
                    prev_key = None
                new.append(inst)
            if n_drop:
                blk.instructions = new
    if os.environ.get("KERNEL_DEBUG"):
        print(f"_dedup_ldweights: dropped {n_drop}")


def kernel(x, W, b, lora_A, lora_B):
    global LAST_RESULT
    from concourse.bass_utils import run_bass_kernel_spmd

    if "nc" not in _cache:
        _cache["nc"] = _build_module()
    nc = _cache["nc"]

    xf = np.ascontiguousarray(x.reshape(_T, _D)).astype(_BF16)
    xT = np.ascontiguousarray(xf.T)                              # [D, T]
    # [D, T] -> [p, ds, T] so each DMA line is contiguous per partition
    xprep = np.ascontiguousarray(xT.reshape(DS, P, _T).transpose(1, 0, 2))
    WT = W.astype(_BF16).T                                       # [D, O]
    # [ds, p, ot, o] -> [ot, p, ds, o] -> [ot*p, ds, o]: 8KB contiguous lines
    Wprep = np.ascontiguousarray(
        WT.reshape(DS, P, NOT, P).transpose(2, 1, 0, 3)
    ).reshape(NOT * P, DS, P)
    ATprep = np.ascontiguousarray(
        lora_A.astype(_BF16).T.reshape(DS, P, _R).transpose(1, 0, 2)
    )
    Baug = np.concatenate(
        [(2.0 * lora_B).astype(_BF16).T, b.astype(_BF16)[None, :]], axis=0
    )  # [17, O]

    in_maps = []
    for c in range(_NCORES):
        t0 = c * _TC
        in_maps.append(
            {
                "x0": np.ascontiguousarray(xprep[:, :, t0 : t0 + TCH]),
                "x1": np.ascontiguousarray(xprep[:, :, t0 + TCH : t0 + 2 * TCH]),
                "Wp": Wprep,
                "ATp": ATprep,
                "Baug": Baug,
                "ones": np.ones((1, _TC), dtype=_BF16),
            }
        )

    trace = os.environ.get("KERNEL_TRACE", "0") == "1"
    res = run_bass_kernel_spmd(
        nc,
        in_maps,
        core_ids=list(range(_NCORES)),
        trace=trace,
    )
    LAST_RESULT = res

    out = np.empty((_T, _O), dtype=np.float32)
    for c, r in enumerate(res.results):
        out[c * _TC : (c + 1) * _TC, :] = r["out"].T
    return out.reshape(_B, _S, _O)
